# revision 27
# baseline (speedup 1.0000x reference)
"""VGAE (3x GCNConv + BN + reparam + MLP decode + dense sigmoid(Z Z^T))
on 8 Trainium2 NeuronCores via Bass/Tile.

Sharding: nodes row-partitioned 8 ways (1500/core). Edge aggregation is
sharded by dst; per-core nodes are degree-sorted into 12 chunks of 128 so
each chunk's padded ELL slot count is near its mean in-degree. Feature
rows are fetched with dma_gather (4 SWDGE queues), aggregated on DVE,
and the two post-hidden GCN layers share a single gather pass since
(A @ h) @ W == A @ (h @ W).  BatchNorm is folded into the next layer's
weights (a*sig+b form) so the pre-BN activations can be AllGathered
concurrently with the stats AllReduce.  The NxN decode is sharded by
output rows; Z^T is assembled from an AllGather of locally transposed
z parts and stays SBUF-resident as the streaming matmul operand.
"""

import numpy as np

import concourse.bass as bass
import concourse.tile as tile
from concourse import bacc, mybir
from concourse import bass_utils
from concourse.masks import make_identity

f32 = mybir.dt.float32
i16 = mybir.dt.int16
i32 = mybir.dt.int32
AF = mybir.ActivationFunctionType
OP = mybir.AluOpType

# Problem constants (hardcoded per harness contract).
N = 12000
F_IN = 128
D = 64
NC = 8
RPC = N // NC          # 1500 rows per core
P = 128
CH = (RPC + P - 1) // P  # 12 chunks per core
SLOTS = CH * P           # 1536 (36 dummy)
NPAD = ((N + P - 1) // P) * P  # 12032
BN_EPS = 1e-4
OOB = 60000


# ----------------------------------------------------------------- host prep

def _preprocess(x, src, dst, noise):
    src = np.asarray(src).astype(np.int64)
    dst = np.asarray(dst).astype(np.int64)
    indeg = np.bincount(dst, minlength=N)
    deg = (indeg + 1).astype(np.float32)
    dis = (1.0 / np.sqrt(deg)).astype(np.float32)

    order = np.argsort(dst, kind="stable")
    s_sorted = src[order]
    rowptr = np.zeros(N + 1, np.int64)
    rowptr[1:] = np.cumsum(indeg)

    perms = []
    table_pos = np.zeros(N, np.int64)
    for c in range(NC):
        ids = np.arange(c * RPC, (c + 1) * RPC)
        order_c = np.argsort(-indeg[ids], kind="stable")
        pids = ids[order_c]
        perm = np.concatenate([pids, np.full(SLOTS - RPC, -1, np.int64)])
        perms.append(perm)
        table_pos[pids] = c * SLOTS + np.arange(RPC)

    # Per-chunk slot count, shared across cores (one SPMD program).
    K = []
    for ch in range(CH):
        m = 1
        for c in range(NC):
            sl = perms[c][ch * P:(ch + 1) * P]
            real = sl[sl >= 0]
            if len(real):
                m = max(m, int(indeg[real].max()) + 1)
        K.append(m)

    def wrap(arr_sm):  # [K, P] slot-major -> [128, K*8] int16
        flat = arr_sm.reshape(-1)
        w = flat.reshape(-1, 16).T
        return np.tile(w, (8, 1)).astype(np.int16)

    per_core = []
    for c in range(NC):
        perm = perms[c]
        idx1_parts, idx2_parts, norm_parts = [], [], []
        rowsum = np.zeros((P, CH), np.float32)
        # dummies scatter into per-partition scratch rows [RPC, RPC+P)
        zidx = np.tile((RPC + np.arange(P, dtype=np.int32))[:, None], (1, CH))
        for ch in range(CH):
            k = K[ch]
            i1 = np.zeros((k, P), np.int64)
            i2 = np.zeros((k, P), np.int64)
            nt = np.zeros((P, k), np.float32)
            for p in range(P):
                node = perm[ch * P + p]
                if node < 0:
                    continue
                di = indeg[node]
                srcs = s_sorted[rowptr[node]:rowptr[node] + di]
                i1[:di, p] = srcs
                i2[:di, p] = table_pos[srcs]
                nt[p, :di] = dis[srcs] * dis[node]
                i1[di, p] = node
                i2[di, p] = table_pos[node]
                nt[p, di] = dis[node] * dis[node]
                zidx[p, ch] = node - c * RPC
            idx1_parts.append(wrap(i1))
            idx2_parts.append(wrap(i2))
            norm_parts.append(nt)
            rowsum[:, ch] = nt.sum(axis=1)
        noise_p = np.zeros((SLOTS, D), np.float32)
        real = perm >= 0
        noise_p[real] = noise[perm[real]]
        mask = np.zeros((P, 1), np.float32)
        mask[: RPC - (CH - 1) * P, 0] = 1.0
        per_core.append({
            "idx1": np.concatenate(idx1_parts, axis=1),
            "idx2": np.concatenate(idx2_parts, axis=1),
            "normt": np.concatenate(norm_parts, axis=1),
            "rowsum": rowsum,
            "zidx": zidx,
            "noise_p": noise_p,
            "mask": mask,
        })
    return per_core, K


# ------------------------------------------------------------- device program

def _build(K, w, debug=False):
    """K: list of per-chunk slot counts. w: dict of weight arrays (for
    shapes only — actual values staged as inputs)."""
    SK = sum(K)
    nc = bacc.Bacc("TRN2", target_bir_lowering=False, debug=False,
                   enable_asserts=True, num_devices=NC, num_swdge_queues=4)

    # ---- I/O
    xT_in = nc.dram_tensor("xT_in", [F_IN, NPAD], f32, kind="ExternalInput")
    idx1_in = nc.dram_tensor("idx1_in", [P, SK * 8], i16, kind="ExternalInput")
    idx2_in = nc.dram_tensor("idx2_in", [P, SK * 8], i16, kind="ExternalInput")
    norm_in = nc.dram_tensor("norm_in", [P, SK], f32, kind="ExternalInput")
    rowsum_in = nc.dram_tensor("rowsum_in", [P, CH], f32, kind="ExternalInput")
    zidx_in = nc.dram_tensor("zidx_in", [P, CH], i32, kind="ExternalInput")
    noise_in = nc.dram_tensor("noise_in", [SLOTS, D], f32, kind="ExternalInput")
    mask_in = nc.dram_tensor("mask_in", [P, 1], f32, kind="ExternalInput")
    W0_in = nc.dram_tensor("W0_in", [F_IN, D], f32, kind="ExternalInput")
    WmWs_in = nc.dram_tensor("WmWs_in", [D, 2 * D], f32, kind="ExternalInput")
    b0b_in = nc.dram_tensor("b0b_in", [P, D], f32, kind="ExternalInput")
    bias2b_in = nc.dram_tensor("bias2b_in", [P, 2 * D], f32, kind="ExternalInput")
    g0r_in = nc.dram_tensor("g0r_in", [1, D], f32, kind="ExternalInput")
    be0r_in = nc.dram_tensor("be0r_in", [1, D], f32, kind="ExternalInput")
    g23r_in = nc.dram_tensor("g23r_in", [1, 2 * D], f32, kind="ExternalInput")
    be23r_in = nc.dram_tensor("be23r_in", [1, 2 * D], f32, kind="ExternalInput")
    Dw1_in = nc.dram_tensor("Dw1_in", [D, D], f32, kind="ExternalInput")
    Dw2_in = nc.dram_tensor("Dw2_in", [D, F_IN], f32, kind="ExternalInput")
    Db1b_in = nc.dram_tensor("Db1b_in", [P, D], f32, kind="ExternalInput")
    Db2b_in = nc.dram_tensor("Db2b_in", [P, F_IN], f32, kind="ExternalInput")

    a_out = nc.dram_tensor("a_out", [RPC, N], f32, kind="ExternalOutput")
    rx_out = nc.dram_tensor("rx_out", [RPC, F_IN], f32, kind="ExternalOutput")
    if debug:
        dbg_h1 = nc.dram_tensor("dbg_h1", [NPAD, D], f32, kind="ExternalOutput")
        dbg_sigb = nc.dram_tensor("dbg_sigb", [SLOTS, D], f32, kind="ExternalOutput")
        dbg_sigtab = nc.dram_tensor("dbg_sigtab", [NC * SLOTS, D], f32, kind="ExternalOutput")
        dbg_ar1 = nc.dram_tensor("dbg_ar1", [1, 2 * D], f32, kind="ExternalOutput")
        dbg_ar2 = nc.dram_tensor("dbg_ar2", [1, 4 * D], f32, kind="ExternalOutput")
        dbg_z = nc.dram_tensor("dbg_z", [RPC + P, D], f32, kind="ExternalOutput")
        dbg_zttab = nc.dram_tensor("dbg_zttab", [NC * D, RPC], f32, kind="ExternalOutput")

    INV_N = 1.0 / N
    RG = [list(range(NC))]

    with tile.TileContext(nc) as tc:
        with (
            tc.tile_pool(name="big", bufs=1) as bigp,
            tc.tile_pool(name="io", bufs=1) as iop,
            tc.tile_pool(name="gat", bufs=3) as gatp,
            tc.tile_pool(name="bat", bufs=1) as batp,
            tc.tile_pool(name="wrk", bufs=3) as wrkp,
            tc.tile_pool(name="dec", bufs=3) as decp,
            tc.tile_pool(name="pmm", bufs=2, space="PSUM") as pmm,
            tc.tile_pool(name="pst", bufs=1, space="PSUM") as pst,
            tc.tile_pool(name="pdec", bufs=2, space="PSUM") as pdec,
            tc.tile_pool(name="dram", bufs=1, space="DRAM") as dram,
        ):
            # ---- internal DRAM
            h1_tab = dram.tile([NPAD, D], f32)
            sig_bounce = dram.tile([SLOTS, D], f32)
            sig_tab = dram.tile([NC * SLOTS, D], f32, addr_space="Shared")
            ar1_in = dram.tile([1, 2 * D], f32)
            ar1_out = dram.tile([1, 2 * D], f32, addr_space="Shared")
            ar2_in = dram.tile([1, 4 * D], f32)
            ar2_out = dram.tile([1, 4 * D], f32, addr_space="Shared")
            z_bounce = dram.tile([RPC + P, D], f32)  # + scratch rows for dummies
            zT_bounce = dram.tile([D, RPC], f32)
            zT_tab = dram.tile([NC * D, RPC], f32, addr_space="Shared")

            # ---- constants / small inputs
            idx1_sb = iop.tile([P, SK * 8], i16)
            nc.sync.dma_start(out=idx1_sb[:], in_=idx1_in[:, :])
            idx2_sb = iop.tile([P, SK * 8], i16)
            nc.sync.dma_start(out=idx2_sb[:], in_=idx2_in[:, :])
            norm_sb = iop.tile([P, SK], f32)
            nc.sync.dma_start(out=norm_sb[:], in_=norm_in[:, :])
            rowsum_sb = iop.tile([P, CH], f32)
            nc.sync.dma_start(out=rowsum_sb[:], in_=rowsum_in[:, :])
            zidx_sb = iop.tile([P, CH], i32)
            nc.sync.dma_start(out=zidx_sb[:], in_=zidx_in[:, :])
            noise_sb = iop.tile([P, CH, D], f32)
            nc.sync.dma_start(
                out=noise_sb[:],
                in_=noise_in[:, :].rearrange("(c p) d -> p c d", p=P))
            mask_sb = iop.tile([P, 1], f32)
            nc.sync.dma_start(out=mask_sb[:], in_=mask_in[:, :])
            W0_sb = iop.tile([F_IN, D], f32)
            nc.sync.dma_start(out=W0_sb[:], in_=W0_in[:, :])
            WmWs_sb = iop.tile([D, 2 * D], f32)
            nc.sync.dma_start(out=WmWs_sb[:], in_=WmWs_in[:, :])
            b0b_sb = iop.tile([P, D], f32)
            nc.sync.dma_start(out=b0b_sb[:], in_=b0b_in[:, :])
            bias2b_sb = iop.tile([P, 2 * D], f32)
            nc.sync.dma_start(out=bias2b_sb[:], in_=bias2b_in[:, :])
            g0r_sb = iop.tile([1, D], f32)
            nc.sync.dma_start(out=g0r_sb[:], in_=g0r_in[:, :])
            be0r_sb = iop.tile([1, D], f32)
            nc.sync.dma_start(out=be0r_sb[:], in_=be0r_in[:, :])
            g23r_sb = iop.tile([1, 2 * D], f32)
            nc.sync.dma_start(out=g23r_sb[:], in_=g23r_in[:, :])
            be23r_sb = iop.tile([1, 2 * D], f32)
            nc.sync.dma_start(out=be23r_sb[:], in_=be23r_in[:, :])
            Dw1_sb = iop.tile([D, D], f32)
            nc.sync.dma_start(out=Dw1_sb[:], in_=Dw1_in[:, :])
            Dw2_sb = iop.tile([D, F_IN], f32)
            nc.sync.dma_start(out=Dw2_sb[:], in_=Dw2_in[:, :])
            Db1b_sb = iop.tile([P, D], f32)
            nc.sync.dma_start(out=Db1b_sb[:], in_=Db1b_in[:, :])
            Db2b_sb = iop.tile([P, F_IN], f32)
            nc.sync.dma_start(out=Db2b_sb[:], in_=Db2b_in[:, :])

            ident = iop.tile([P, P], f32)
            make_identity(nc, ident[:])
            ones_col = iop.tile([P, 1], f32)
            nc.vector.memset(ones_col[:], 1.0)
            ones_row = iop.tile([1, P], f32)
            nc.vector.memset(ones_row[:], 1.0)
            ones11 = iop.tile([1, 1], f32)
            nc.vector.memset(ones11[:], 1.0)
            eps11 = iop.tile([1, 1], f32)
            nc.vector.memset(eps11[:], BN_EPS)

            # ================= PHASE 1: H1 = x @ W0 (replicated, orig order)
            xT_sb = bigp.tile([F_IN, NPAD], f32, tag="big")
            nc.sync.dma_start(out=xT_sb[:], in_=xT_in[:, :])
            NB = NPAD // P  # 94
            h1_stage = wrkp.tile([P, 8, D], f32, tag="h1s")
            for b in range(NB):
                ph1 = pmm.tile([P, D], f32, tag="mm")
                nc.tensor.matmul(ph1[:], lhsT=xT_sb[:, b * P:(b + 1) * P],
                                 rhs=W0_sb[:], start=True, stop=True)
                g8 = b % 8
                nc.vector.tensor_copy(out=h1_stage[:, g8, :], in_=ph1[:])
                if g8 == 7 or b == NB - 1:
                    lo = (b - g8) * P
                    hi = (b + 1) * P
                    nc.sync.dma_start(
                        out=h1_tab[lo:hi, :].rearrange("(c p) d -> p c d", p=P),
                        in_=h1_stage[:, :g8 + 1, :])
                    if b != NB - 1:
                        h1_stage = wrkp.tile([P, 8, D], f32, tag="h1s")

            # ================= PHASE 2: L1 aggregation over edges
            pre1 = batp.tile([P, CH, D], f32)
            off = 0
            for ch in range(CH):
                k = K[ch]
                ni = k * P
                g = gatp.tile([P, K[0], D], f32, tag="g")
                nc.gpsimd.dma_gather(
                    out_ap=g[:, :k, :], in_ap=h1_tab[:],
                    idxs_ap=idx1_sb[:, off * 8:(off + k) * 8],
                    num_idxs=ni, num_idxs_reg=ni, elem_size=D,
                    single_packet=False, queue_num=ch % 4)
                nc.vector.tensor_tensor(
                    out=g[:, :k, :], in0=g[:, :k, :],
                    in1=norm_sb[:, off:off + k].to_broadcast([P, k, D]),
                    op=OP.mult)
                nc.vector.tensor_reduce(
                    out=pre1[:, ch, :], in_=g[:, :k, :].rearrange("p k d -> p d k"),
                    axis=mybir.AxisListType.X, op=OP.add)
                off += k

            nc.vector.tensor_tensor(
                out=pre1[:], in0=pre1[:],
                in1=b0b_sb[:, None, :].to_broadcast([P, CH, D]), op=OP.add)
            # packed [sig | sig^2] per chunk so stats are ONE psum group
            ssq1 = batp.tile([P, CH, 2 * D], f32)
            nc.scalar.activation(out=ssq1[:, :, 0:D], in_=pre1[:],
                                 func=AF.Sigmoid)
            nc.vector.tensor_tensor(out=ssq1[:, :, D:2 * D],
                                    in0=ssq1[:, :, 0:D],
                                    in1=ssq1[:, :, 0:D], op=OP.mult)
            # pre-BN table out (AllGather) + masked stats (AllReduce), overlap
            nc.sync.dma_start(
                out=sig_bounce[:].rearrange("(c p) d -> p c d", p=P),
                in_=ssq1[:, :, 0:D])
            st1 = pst.tile([1, 2 * D], f32, tag="st")
            for ch in range(CH):
                lhsT = mask_sb if ch == CH - 1 else ones_col
                nc.tensor.matmul(st1[:], lhsT=lhsT[:], rhs=ssq1[:, ch, :],
                                 start=(ch == 0), stop=(ch == CH - 1))
            st1_sb = wrkp.tile([1, 2 * D], f32, tag="sts")
            nc.vector.tensor_copy(out=st1_sb[:], in_=st1[:])
            nc.sync.dma_start(out=ar1_in[:], in_=st1_sb[:])
            nc.gpsimd.collective_compute(
                "AllGather", OP.bypass, replica_groups=RG,
                ins=[sig_bounce[:].opt()], outs=[sig_tab[:].opt()])
            nc.gpsimd.collective_compute(
                "AllReduce", OP.add, replica_groups=RG,
                ins=[ar1_in[:].opt()], outs=[ar1_out[:].opt()])

            # ================= PHASE 3: BN1 fold into WmWs
            # row-space math on [1, D] slices of the AllReduced stats
            s1 = wrkp.tile([1, 2 * D], f32, tag="sts")
            nc.sync.dma_start(out=s1[:], in_=ar1_out[:])
            mu1 = wrkp.tile([1, D], f32, tag="c1")
            nc.vector.tensor_scalar_mul(mu1[:], s1[:, 0:D], INV_N)
            var1 = wrkp.tile([1, D], f32, tag="c2")
            nc.vector.tensor_scalar_mul(var1[:], s1[:, D:2 * D], INV_N)
            mu1sq = wrkp.tile([1, D], f32, tag="c3")
            nc.vector.tensor_tensor(mu1sq[:], mu1[:], mu1[:], op=OP.mult)
            nc.vector.tensor_tensor(var1[:], var1[:], mu1sq[:], op=OP.subtract)
            nc.scalar.activation(out=var1[:], in_=var1[:], func=AF.Sqrt,
                                 bias=eps11[:], scale=1.0)
            nc.vector.reciprocal(out=var1[:], in_=var1[:])
            ab1 = wrkp.tile([1, 2 * D], f32, tag="c4")
            nc.vector.tensor_tensor(ab1[:, 0:D], g0r_sb[:], var1[:],
                                    op=OP.mult)           # a1
            nc.vector.tensor_tensor(ab1[:, D:2 * D], mu1[:], ab1[:, 0:D],
                                    op=OP.mult)
            nc.vector.tensor_tensor(ab1[:, D:2 * D], be0r_sb[:],
                                    ab1[:, D:2 * D], op=OP.subtract)  # b1
            # transpose [1, D] rows -> [D, 1] cols via K=1 matmuls (base 0)
            pa1c = pmm.tile([D, 1], f32, tag="mm")
            nc.tensor.matmul(pa1c[:], lhsT=ab1[:, 0:D], rhs=ones11[:],
                             start=True, stop=True)
            a1c = wrkp.tile([D, 1], f32, tag="c5")
            nc.vector.tensor_copy(out=a1c[:], in_=pa1c[:])
            pb1c = pmm.tile([D, 1], f32, tag="mm")
            nc.tensor.matmul(pb1c[:], lhsT=ab1[:, D:2 * D], rhs=ones11[:],
                             start=True, stop=True)
            b1c = wrkp.tile([D, 1], f32, tag="c6")
            nc.vector.tensor_copy(out=b1c[:], in_=pb1c[:])
            WmWs_f = iop.tile([D, 2 * D], f32)
            nc.vector.tensor_scalar_mul(WmWs_f[:], WmWs_sb[:], a1c[:])
            pbW = pmm.tile([1, 2 * D], f32, tag="mm")
            nc.tensor.matmul(pbW[:], lhsT=b1c[:], rhs=WmWs_sb[:],
                             start=True, stop=True)
            bW_row = wrkp.tile([1, 2 * D], f32, tag="r1")
            nc.vector.tensor_copy(out=bW_row[:], in_=pbW[:])
            pbWb = pmm.tile([P, 2 * D], f32, tag="mm")
            nc.tensor.matmul(pbWb[:], lhsT=ones_row[:], rhs=bW_row[:],
                             start=True, stop=True)
            bWb = iop.tile([P, 2 * D], f32)
            nc.vector.tensor_copy(out=bWb[:], in_=pbWb[:])

            # ================= PHASE 4: L2/L3 shared aggregation
            pre2 = batp.tile([P, CH, 2 * D], f32)
            off = 0
            for ch in range(CH):
                k = K[ch]
                ni = k * P
                g2 = gatp.tile([P, K[0], D], f32, tag="g")
                nc.gpsimd.dma_gather(
                    out_ap=g2[:, :k, :], in_ap=sig_tab[:],
                    idxs_ap=idx2_sb[:, off * 8:(off + k) * 8],
                    num_idxs=ni, num_idxs_reg=ni, elem_size=D,
                    single_packet=False, queue_num=ch % 4)
                nc.vector.tensor_tensor(
                    out=g2[:, :k, :], in0=g2[:, :k, :],
                    in1=norm_sb[:, off:off + k].to_broadcast([P, k, D]),
                    op=OP.mult)
                aggs = wrkp.tile([P, D], f32, tag="aggs")
                nc.vector.tensor_reduce(
                    out=aggs[:], in_=g2[:, :k, :].rearrange("p k d -> p d k"),
                    axis=mybir.AxisListType.X, op=OP.add)
                pT = pmm.tile([D, P], f32, tag="mm")
                nc.tensor.transpose(pT[:], in_=aggs[:], identity=ident[:])
                aggsT = wrkp.tile([D, P], f32, tag="aggsT")
                nc.vector.tensor_copy(out=aggsT[:], in_=pT[:])
                pmm2 = pmm.tile([P, 2 * D], f32, tag="mm")
                nc.tensor.matmul(pmm2[:], lhsT=aggsT[:], rhs=WmWs_f[:],
                                 start=True, stop=True)
                nc.vector.scalar_tensor_tensor(
                    out=pre2[:, ch, :], in0=bWb[:],
                    scalar=rowsum_sb[:, ch:ch + 1], in1=pmm2[:],
                    op0=OP.mult, op1=OP.add)
                off += k

            nc.vector.tensor_tensor(
                out=pre2[:], in0=pre2[:],
                in1=bias2b_sb[:, None, :].to_broadcast([P, CH, 2 * D]),
                op=OP.add)
            # packed [sig_m | sig_s | sq_m | sq_s] per chunk
            ssq2 = batp.tile([P, CH, 4 * D], f32)
            nc.scalar.activation(out=ssq2[:, :, 0:2 * D], in_=pre2[:],
                                 func=AF.Sigmoid)
            nc.vector.tensor_tensor(out=ssq2[:, :, 2 * D:4 * D],
                                    in0=ssq2[:, :, 0:2 * D],
                                    in1=ssq2[:, :, 0:2 * D], op=OP.mult)
            st2 = pst.tile([1, 4 * D], f32, tag="st")
            for ch in range(CH):
                lhsT = mask_sb if ch == CH - 1 else ones_col
                nc.tensor.matmul(st2[:], lhsT=lhsT[:], rhs=ssq2[:, ch, :],
                                 start=(ch == 0), stop=(ch == CH - 1))
            st2_sb = wrkp.tile([1, 4 * D], f32, tag="sts4")
            nc.vector.tensor_copy(out=st2_sb[:], in_=st2[:])
            nc.sync.dma_start(out=ar2_in[:], in_=st2_sb[:])
            nc.gpsimd.collective_compute(
                "AllReduce", OP.add, replica_groups=RG,
                ins=[ar2_in[:].opt()], outs=[ar2_out[:].opt()])

            # ================= PHASE 5: BN2/3 + z (row-space math)
            s2 = wrkp.tile([1, 4 * D], f32, tag="sts4")
            nc.sync.dma_start(out=s2[:], in_=ar2_out[:])
            mu23 = wrkp.tile([1, 2 * D], f32, tag="d1")
            nc.vector.tensor_scalar_mul(mu23[:], s2[:, 0:2 * D], INV_N)
            var23 = wrkp.tile([1, 2 * D], f32, tag="d2")
            nc.vector.tensor_scalar_mul(var23[:], s2[:, 2 * D:4 * D], INV_N)
            mu23sq = wrkp.tile([1, 2 * D], f32, tag="d3")
            nc.vector.tensor_tensor(mu23sq[:], mu23[:], mu23[:], op=OP.mult)
            nc.vector.tensor_tensor(var23[:], var23[:], mu23sq[:],
                                    op=OP.subtract)
            nc.scalar.activation(out=var23[:], in_=var23[:], func=AF.Sqrt,
                                 bias=eps11[:], scale=1.0)
            nc.vector.reciprocal(out=var23[:], in_=var23[:])
            # t4 row = [a_m | a_s | b_m | b_s]
            t4 = wrkp.tile([1, 4 * D], f32, tag="sts4b")
            nc.vector.tensor_tensor(t4[:, 0:2 * D], g23r_sb[:], var23[:],
                                    op=OP.mult)
            nc.vector.tensor_tensor(t4[:, 2 * D:4 * D], mu23[:],
                                    t4[:, 0:2 * D], op=OP.mult)
            nc.vector.tensor_tensor(t4[:, 2 * D:4 * D], be23r_sb[:],
                                    t4[:, 2 * D:4 * D], op=OP.subtract)
            # broadcast all four rows at once: [128, 4D]
            pab = pmm.tile([P, 4 * D], f32, tag="mm")
            nc.tensor.matmul(pab[:], lhsT=ones_row[:], rhs=t4[:],
                             start=True, stop=True)
            ab = iop.tile([P, 4 * D], f32)
            nc.vector.tensor_copy(out=ab[:], in_=pab[:])

            t_all = batp.tile([P, CH, D], f32)
            nc.vector.tensor_tensor(
                out=t_all[:], in0=ssq2[:, :, D:2 * D],
                in1=ab[:, D:2 * D][:, None, :].to_broadcast([P, CH, D]),
                op=OP.mult)
            nc.vector.tensor_tensor(
                out=t_all[:], in0=t_all[:],
                in1=ab[:, 3 * D:4 * D][:, None, :].to_broadcast([P, CH, D]),
                op=OP.add)
            e_all = batp.tile([P, CH, D], f32, tag="ssq1")
            nc.scalar.activation(out=e_all[:], in_=t_all[:], func=AF.Exp)
            nc.vector.tensor_tensor(out=e_all[:], in0=e_all[:],
                                    in1=noise_sb[:], op=OP.mult)
            m_all = batp.tile([P, CH, D], f32, tag="pre1")
            nc.vector.tensor_tensor(
                out=m_all[:], in0=ssq2[:, :, 0:D],
                in1=ab[:, 0:D][:, None, :].to_broadcast([P, CH, D]),
                op=OP.mult)
            nc.vector.tensor_tensor(
                out=m_all[:], in0=m_all[:],
                in1=ab[:, 2 * D:3 * D][:, None, :].to_broadcast([P, CH, D]),
                op=OP.add)
            z_all = batp.tile([P, CH, D], f32, tag="tall2")
            nc.vector.tensor_tensor(out=z_all[:], in0=e_all[:], in1=m_all[:],
                                    op=OP.add)

            # scatter z to original order (dummy slots land in scratch rows)
            for ch in range(CH):
                nc.gpsimd.indirect_dma_start(
                    out=z_bounce[:], out_offset=bass.IndirectOffsetOnAxis(
                        ap=zidx_sb[:, ch:ch + 1], axis=0),
                    in_=z_all[:, ch, :], in_offset=None)

            # ================= PHASE 6: local z^T + rx MLP + AllGather z^T
            zTb = bigp.tile([D, RPC], f32, tag="ztb")
            for ch in range(CH):
                rows = min(P, RPC - ch * P)
                zr = wrkp.tile([P, D], f32, tag="zr")
                nc.sync.dma_start(out=zr[:rows, :],
                                  in_=z_bounce[ch * P:ch * P + rows, :])
                pzT = pmm.tile([D, P], f32, tag="mm")
                nc.tensor.transpose(pzT[:, :rows], in_=zr[:rows, :],
                                    identity=ident[:rows, :rows])
                nc.vector.tensor_copy(out=zTb[:, ch * P:ch * P + rows],
                                      in_=pzT[:, :rows])
                # rx MLP on this chunk (lhsT = zTb slice)
                ph = pmm.tile([P, D], f32, tag="mm")
                nc.tensor.matmul(ph[:rows, :],
                                 lhsT=zTb[:, ch * P:ch * P + rows],
                                 rhs=Dw1_sb[:], start=True, stop=True)
                hpre = wrkp.tile([P, D], f32, tag="hpre")
                nc.vector.tensor_tensor(out=hpre[:rows, :], in0=ph[:rows, :],
                                        in1=Db1b_sb[:rows, :], op=OP.add)
                hact = wrkp.tile([P, D], f32, tag="hact")
                nc.scalar.activation(out=hact[:rows, :], in_=hpre[:rows, :],
                                     func=AF.Lrelu, alpha=0.01)
                phT = pmm.tile([D, P], f32, tag="mm")
                nc.tensor.transpose(phT[:, :rows], in_=hact[:rows, :],
                                    identity=ident[:rows, :rows])
                hT = wrkp.tile([D, P], f32, tag="hT")
                nc.vector.tensor_copy(out=hT[:, :rows], in_=phT[:, :rows])
                prx = pmm.tile([P, F_IN], f32, tag="mm")
                nc.tensor.matmul(prx[:rows, :], lhsT=hT[:, :rows],
                                 rhs=Dw2_sb[:], start=True, stop=True)
                rx_sb = wrkp.tile([P, F_IN], f32, tag="rx")
                nc.vector.tensor_tensor(out=rx_sb[:rows, :], in0=prx[:rows, :],
                                        in1=Db2b_sb[:rows, :], op=OP.add)
                nc.sync.dma_start(out=rx_out[ch * P:ch * P + rows, :],
                                  in_=rx_sb[:rows, :])

            nc.sync.dma_start(out=zT_bounce[:], in_=zTb[:])
            nc.gpsimd.collective_compute(
                "AllGather", OP.bypass, replica_groups=RG,
                ins=[zT_bounce[:].opt()], outs=[zT_tab[:].opt()])

            # ================= PHASE 7: decode A = sigmoid(Z_local @ Z^T)
            ZT = bigp.tile([D, N], f32, tag="big")
            for c in range(NC):
                nc.sync.dma_start(out=ZT[:, c * RPC:(c + 1) * RPC],
                                  in_=zT_tab[c * D:(c + 1) * D, :])
            CB = 1024
            for ch in range(CH):
                rows = min(P, RPC - ch * P)
                lhsT = zTb[:, ch * P:ch * P + rows]
                for c0 in range(0, N, CB):
                    cw = min(CB, N - c0)
                    pd = pdec.tile([P, CB], f32, tag="pd")
                    for s0 in range(0, cw, 512):
                        sw = min(512, cw - s0)
                        nc.tensor.matmul(
                            pd[:rows, s0:s0 + sw], lhsT=lhsT,
                            rhs=ZT[:, c0 + s0:c0 + s0 + sw],
                            start=True, stop=True)
                    asb = decp.tile([P, CB], f32, tag="asb")
                    nc.scalar.activation(out=asb[:rows, :cw],
                                         in_=pd[:rows, :cw], func=AF.Sigmoid)
                    nc.sync.dma_start(
                        out=a_out[ch * P:ch * P + rows, c0:c0 + cw],
                        in_=asb[:rows, :cw])

            if debug:
                def _dump(dst_t, src_t, rows, width):
                    nb = (rows + P - 1) // P
                    for b in range(nb):
                        r = min(P, rows - b * P)
                        tmp = decp.tile([P, width], f32, tag="dbg", bufs=1)
                        nc.sync.dma_start(out=tmp[:r, :width],
                                          in_=src_t[b * P:b * P + r, :])
                        nc.sync.dma_start(out=dst_t[b * P:b * P + r, :],
                                          in_=tmp[:r, :width])
                import os as _os
                _sel = _os.environ.get("VGAE_DBG", "1234567")
                if "1" in _sel: _dump(dbg_h1, h1_tab, NPAD, D)
                if "2" in _sel: _dump(dbg_sigb, sig_bounce, SLOTS, D)
                if "3" in _sel: _dump(dbg_sigtab, sig_tab, NC * SLOTS, D)
                if "4" in _sel: _dump(dbg_ar1, ar1_out, 1, 2 * D)
                if "5" in _sel: _dump(dbg_ar2, ar2_out, 1, 4 * D)
                if "6" in _sel: _dump(dbg_z, z_bounce, RPC + P, D)
                if "7" in _sel: _dump(dbg_zttab, zT_tab, NC * D, RPC)

    nc.compile()
    return nc


# ------------------------------------------------------------------ wrapper

_CACHE = {}


def kernel(x, src, dst, batch, noise,
           W0, b0, g0, be0,
           Wm, bm, gm, bem,
           Ws, bs, gs, bes,
           Dw1, Db1, Dw2, Db2):
    x = np.asarray(x, np.float32)
    noise = np.asarray(noise, np.float32)
    per_core, K = _preprocess(x, src, dst, noise)

    W0 = np.asarray(W0, np.float32)
    WmWs = np.concatenate([np.asarray(Wm, np.float32),
                           np.asarray(Ws, np.float32)], axis=1)
    xT = np.zeros((F_IN, NPAD), np.float32)
    xT[:, :N] = x.T
    b0b = np.tile(np.asarray(b0, np.float32)[None, :], (P, 1))
    bias2b = np.tile(np.concatenate([np.asarray(bm, np.float32),
                                     np.asarray(bs, np.float32)])[None, :],
                     (P, 1))
    g0r = np.asarray(g0, np.float32)[None, :]
    be0r = np.asarray(be0, np.float32)[None, :]
    g23r = np.concatenate([np.asarray(gm, np.float32),
                           np.asarray(gs, np.float32)])[None, :]
    be23r = np.concatenate([np.asarray(bem, np.float32),
                            np.asarray(bes, np.float32)])[None, :]
    Dw1 = np.asarray(Dw1, np.float32)
    Dw2 = np.asarray(Dw2, np.float32)
    Db1b = np.tile(np.asarray(Db1, np.float32)[None, :], (P, 1))
    Db2b = np.tile(np.asarray(Db2, np.float32)[None, :], (P, 1))

    key = tuple(K)
    if key not in _CACHE:
        _CACHE[key] = _build(K, None)
    nc = _CACHE[key]

    in_maps = []
    for c in range(NC):
        pc = per_core[c]
        in_maps.append({
            "xT_in": xT, "idx1_in": pc["idx1"], "idx2_in": pc["idx2"],
            "norm_in": pc["normt"], "rowsum_in": pc["rowsum"],
            "zidx_in": pc["zidx"], "noise_in": pc["noise_p"],
            "mask_in": pc["mask"],
            "W0_in": W0, "WmWs_in": WmWs, "b0b_in": b0b,
            "bias2b_in": bias2b, "g0r_in": g0r, "be0r_in": be0r,
            "g23r_in": g23r, "be23r_in": be23r,
            "Dw1_in": Dw1, "Dw2_in": Dw2, "Db1b_in": Db1b, "Db2b_in": Db2b,
        })

    import os
    trace = bool(int(os.environ.get("VGAE_TRACE", "0")))
    res = bass_utils.run_bass_kernel_spmd(
        nc, in_maps, core_ids=list(range(NC)), trace=trace)
    kernel.last_result = res

    A_pred = np.concatenate([res.results[c]["a_out"] for c in range(NC)], 0)
    rx = np.concatenate([res.results[c]["rx_out"] for c in range(NC)], 0)
    return (rx, A_pred)
